# revision 22
# baseline (speedup 1.0000x reference)
"""MLA attention kernel for TRN2, SPMD over 8 NeuronCores.

Sharding: core c = 4*b + g  (b = batch 0..1, g = head-group 0..3, 4 heads each).
Each core computes, for its batch b and head-group g:
    qT = (Wq_g*scale)^T x^T + bq_g*scale        [256, 2048]   (bf16)
    latT = Wl^T x^T + bl                        [256, 2048]
    kT = Wk_g^T latT                            [256, 2048]   (bk dropped: softmax shift-invariant)
    v  = latT^T Wv_g                            [2048, 256]   (bv folded into host const)
    per head h: sT = kT_h^T qT_h ; pT = exp(sT) (no max-subtraction; scores ~ N(0,1))
                l = 1^T pT ; oT = v_h^T pT ; aT = oT / l
    partial = A Wo_g                            [2048, 1024]  (f32)
Host sums the 4 partials per batch and adds (bv @ Wo + bo).

Scheduling: a slot-gated work queue keeps the scalar engine (exp ACTs, the
~154us floor) continuously fed: every QK "slot" first drains ready filler
chunks (PV, row-sum L matmuls, projections, Wo) whose inputs were emitted
>=2 slots earlier, then emits the next QK pair + exp ACTs.
PSUM: s-tiles 2bufs x 2banks, ot 2 x 1, L 1, misc 1 = 8 banks.
"""
import contextlib
import ctypes
import os
import sys
import types

if "/opt/trn_rl_repo" not in sys.path:
    sys.path.insert(0, "/opt/trn_rl_repo")

import numpy as np
import ml_dtypes

NPBF16 = ml_dtypes.bfloat16
SCALE = 64 ** -0.5
_STATE = {}


# ---------------------------------------------------------------- ntff shim
def _install_ntff_shim():
    """Provide antenv.axon_hooks so run_bass_kernel_spmd(trace=True) works."""
    if "antenv.axon_hooks" in sys.modules:
        return
    try:
        import antenv
    except ImportError:
        return

    so_path = "/opt/axon/libaxon_pjrt.so"

    def _hook_factory():
        try:
            lib = ctypes.CDLL(so_path)
        except OSError:
            return None
        if not hasattr(lib, "axon_start_nrt_profile"):
            return None
        lib.axon_start_nrt_profile.argtypes = [ctypes.POINTER(ctypes.c_int64), ctypes.c_size_t]
        lib.axon_start_nrt_profile.restype = ctypes.c_int64
        lib.axon_stop_nrt_profile.argtypes = [ctypes.c_char_p]
        lib.axon_stop_nrt_profile.restype = ctypes.c_int64

        @contextlib.contextmanager
        def _hook(output_dir, device_ids):
            import jax

            jax.devices()
            if device_ids:
                ids = (ctypes.c_int64 * len(device_ids))(*device_ids)
                rc = lib.axon_start_nrt_profile(ids, len(device_ids))
            else:
                rc = lib.axon_start_nrt_profile(None, 0)
            if rc != 0:
                raise RuntimeError(f"axon_start_nrt_profile rc={rc}")
            try:
                yield
            finally:
                n = lib.axon_stop_nrt_profile(str(output_dir).encode())
                print(f"profile: {n} file(s) written to {output_dir}", file=sys.stderr)

        return _hook

    import antenv

    mod = types.ModuleType("antenv.axon_hooks")
    _state = {"hook": _hook_factory()}
    mod.set_axon_ntff_profile_hook = lambda h: _state.__setitem__("hook", h)
    mod.get_axon_ntff_profile_hook = lambda: _state["hook"]
    sys.modules["antenv.axon_hooks"] = mod
    antenv.axon_hooks = mod


# ---------------------------------------------------------------- bass build
def _build_nc(debug_dump=False):
    import concourse.bass as bass  # noqa: F401
    import concourse.tile as tile
    from concourse import bacc, mybir

    BF16 = mybir.dt.bfloat16
    F32 = mybir.dt.float32
    EXP = mybir.ActivationFunctionType.Exp

    nc = bacc.Bacc(None, target_bir_lowering=False, debug=False)

    xT = nc.dram_tensor("xT", [128, 8, 2048], BF16, kind="ExternalInput")
    wq = nc.dram_tensor("wq", [128, 8, 256], BF16, kind="ExternalInput")
    bq = nc.dram_tensor("bq", [128, 2], F32, kind="ExternalInput")
    wl = nc.dram_tensor("wl", [128, 8, 256], BF16, kind="ExternalInput")
    bl = nc.dram_tensor("bl", [128, 2], F32, kind="ExternalInput")
    wk = nc.dram_tensor("wk", [128, 2, 256], BF16, kind="ExternalInput")
    wv = nc.dram_tensor("wv", [128, 2, 256], BF16, kind="ExternalInput")
    wo = nc.dram_tensor("wo", [128, 2, 1024], BF16, kind="ExternalInput")
    out = nc.dram_tensor("out", [2048, 1024], BF16, kind="ExternalOutput")

    GROUPS = [(2 * i, 2 * i + 2) for i in range(8)]
    GLEN = 2

    with nc.allow_low_precision("bf16 intermediates by design"), tile.TileContext(nc) as tc:
        with (
            tc.tile_pool(name="wpool", bufs=1) as wpool,
            tc.tile_pool(name="xpool", bufs=1) as xpool,
            tc.tile_pool(name="proj", bufs=1) as proj,
            tc.tile_pool(name="ptp", bufs=24) as ptp,
            tc.tile_pool(name="atp", bufs=4) as atp,
            tc.tile_pool(name="obp", bufs=4) as obp,
            tc.tile_pool(name="rpool", bufs=4) as rpool,
            tc.tile_pool(name="ps", bufs=2, space="PSUM") as ps,
        ):
            # ---------------- constants + inputs
            x_kn = [
                [xpool.tile([128, 512], BF16, name=f"x_{k}_{n}") for n in range(4)]
                for k in range(8)
            ]
            wq_sb = wpool.tile([128, 8, 256], BF16)
            wl_sb = wpool.tile([128, 8, 256], BF16)
            wk_sb = wpool.tile([128, 2, 256], BF16)
            wv_sb = wpool.tile([128, 2, 256], BF16)
            wo_sb = wpool.tile([128, 2, 1024], BF16)
            bq_sb = wpool.tile([128, 2], F32)
            bl_sb = wpool.tile([128, 2], F32)
            ones_sb = wpool.tile([128, 1], BF16)
            ones_k1 = wpool.tile([128, 64], BF16)
            nc.vector.memset(ones_sb[:], 1.0)
            nc.vector.memset(ones_k1[:], 1.0)

            # DMA order: gate the prologue (lat0/kt0/qT0) as early as possible.
            nc.sync.dma_start(out=wl_sb[:], in_=wl[:])
            nc.sync.dma_start(out=bl_sb[:], in_=bl[:])
            for k in range(8):
                nc.sync.dma_start(out=x_kn[k][0][:], in_=xT[:, k, 0:512])
            nc.sync.dma_start(out=wq_sb[:], in_=wq[:])
            nc.sync.dma_start(out=bq_sb[:], in_=bq[:])
            nc.sync.dma_start(out=wk_sb[:], in_=wk[:])
            nc.sync.dma_start(out=wv_sb[:], in_=wv[:])
            for n in range(1, 4):
                for k in range(8):
                    nc.sync.dma_start(
                        out=x_kn[k][n][:],
                        in_=xT[:, k, 512 * n : 512 * n + 512],
                    )
            nc.sync.dma_start(out=wo_sb[:], in_=wo[:])

            latT_n = [proj.tile([128, 2, 512], BF16, name=f"latT_{i}") for i in range(4)]
            qT_n = [proj.tile([128, 2, 512], BF16, name=f"qT_{i}") for i in range(4)]
            kT_n = [proj.tile([128, 2, 512], BF16, name=f"kT_{i}") for i in range(4)]
            v_sb = proj.tile([128, 16, 256], BF16)

            def s_ps(name):
                return ps.tile([128, GLEN, 512], F32, tag="s", name=name, bufs=2)

            def ot_ps(name):
                return ps.tile([128, 512], F32, tag="ot", name=name, bufs=2)

            def L_ps(name):
                return ps.tile([128, 512], F32, tag="L", name=name, bufs=1)

            def misc_ps(name):
                return ps.tile([128, 512], F32, tag="m", name=name, bufs=1)

            # HAM warm-up: small-N dummy matmuls while input DMA is in flight
            warm_sb = wpool.tile([128, 128], BF16)
            nc.vector.memset(warm_sb[:], 0.25)
            warm_ps = misc_ps("warm_ps")
            for i in range(40):
                nc.tensor.matmul(
                    warm_ps[:, 0:128], warm_sb[:], warm_sb[:],
                    start=(i == 0), stop=(i == 39),
                )

            # ---------------- slot-gated work queue
            slot = [0]
            workq = []  # (ready_slot, fn), FIFO

            def enq(ready, fn, cost=4):
                workq.append((ready, fn, cost))

            def pump(drain=False, budget=12):
                i = 0
                spent = 0
                while i < len(workq):
                    ready, fn, cost = workq[i]
                    if drain or (ready <= slot[0] and spent < budget):
                        workq.pop(i)
                        fn()
                        spent += cost
                    else:
                        i += 1

            # ---------------- projection emitters
            def emit_lat(n, m):
                acc = misc_ps(f"lat_ps_{m}_{n}")
                for k in range(8):
                    nc.tensor.matmul(
                        acc[:],
                        wl_sb[:, k, 128 * m : 128 * m + 128],
                        x_kn[k][n][:],
                        start=(k == 0),
                        stop=(k == 7),
                    )
                nc.vector.tensor_scalar_add(
                    out=latT_n[n][:, m, :], in0=acc[:], scalar1=bl_sb[:, m : m + 1]
                )

            def emit_kt(n, m):
                acc = misc_ps(f"kt_ps_{m}_{n}")
                for k in range(2):
                    nc.tensor.matmul(
                        acc[:],
                        wk_sb[:, k, 128 * m : 128 * m + 128],
                        latT_n[n][:, k, :],
                        start=(k == 0),
                        stop=(k == 1),
                    )
                nc.vector.tensor_copy(out=kT_n[n][:, m, :], in_=acc[:])

            def emit_v(ts):
                for t in ts:
                    acc = misc_ps(f"v_ps_{t}")
                    for k in range(2):
                        nc.tensor.matmul(
                            acc[:, 0:256],
                            latT_n[t // 4][:, k, 128 * (t % 4) : 128 * (t % 4) + 128],
                            wv_sb[:, k, :],
                            start=(k == 0),
                            stop=(k == 1),
                        )
                    nc.vector.tensor_copy(out=v_sb[:, t, :], in_=acc[:, 0:256])

            def emit_qt(n, m):
                acc = misc_ps(f"q_ps_{m}_{n}")
                for k in range(8):
                    nc.tensor.matmul(
                        acc[:],
                        wq_sb[:, k, 128 * m : 128 * m + 128],
                        x_kn[k][n][:],
                        start=(k == 0),
                        stop=(k == 7),
                    )
                nc.vector.tensor_scalar_add(
                    out=qT_n[n][:, m, :], in0=acc[:], scalar1=bq_sb[:, m : m + 1]
                )

            # ---------------- attention pieces
            ots = {}   # (ic, p) -> ot psum tile
            Ls = {}    # ic -> L psum tile
            ats = {}   # ic -> {p: at tile}

            def emit_pv_chunk(ic, p, gi, pta, ptb):
                """pta/ptb = pt tiles for key-chunks t0/t1, each [128, 2heads, 512]."""
                h0, h1 = 2 * p, 2 * p + 1
                if gi == 0:
                    ots[(ic, p)] = ot_ps(f"ot_{ic}_{p}")
                ot0 = ots[(ic, p)]
                t0, t1 = GROUPS[gi]
                for t in range(t0, t1):
                    pt = pta if t == t0 else ptb
                    nc.tensor.matmul(
                        ot0[0:64, :], v_sb[:, t, 64 * h0 : 64 * h0 + 64], pt[:, 0, :],
                        start=(t == 0), stop=(t == 15), skip_group_check=True,
                    )
                    nc.tensor.matmul(
                        ot0[64:128, :], v_sb[:, t, 64 * h1 : 64 * h1 + 64], pt[:, 1, :],
                        start=(t == 0), stop=(t == 15), skip_group_check=True,
                    )

            def emit_sums_chunk(ic, gi, pts):
                """L row-sums for all 4 heads (both pairs) of group gi —
                foursomes of col tiles issue concurrently."""
                if gi == 0:
                    L = L_ps(f"L_{ic}")
                    nc.vector.memset(L[:], 1.0)
                    Ls[ic] = L
                L = Ls[ic]
                t0, t1 = GROUPS[gi]
                for t in range(t0, t1):
                    tt = t - t0
                    for p in range(2):
                        pta, ptb = pts[(ic, p, gi)]
                        pt = pta if t == t0 else ptb
                        for j, hh in enumerate((2 * p, 2 * p + 1)):
                            nc.tensor.matmul(
                                L[32 * hh : 32 * hh + 1, :],
                                ones_sb[:],
                                pt[:, j, :],
                                start=(t == 0),
                                stop=(t == 15),
                                tile_position=(0, 32 * hh),
                                skip_group_check=True,
                            )
                if gi == 7:
                    for p in range(2):
                        for g2 in range(8):
                            pts.pop((ic, p, g2))

            def emit_norm(ic):
                L = Ls.pop(ic)
                recip = rpool.tile([128, 512], BF16, tag="recip", name=f"recip_{ic}")
                nc.vector.reciprocal(out=recip[:], in_=L[:])
                pair_ats = {}
                for p in range(2):
                    bc_ps = misc_ps(f"bcp_{ic}_{p}")
                    for j, hh in enumerate((2 * p, 2 * p + 1)):
                        rb = 32 * hh
                        nc.tensor.matmul(
                            bc_ps[64 * j : 64 * j + 64, :],
                            ones_k1[rb : rb + 1, 0:64],
                            recip[rb : rb + 1, :],
                            start=True,
                            stop=True,
                            tile_position=(rb, 64 * j),
                            skip_group_check=True,
                        )
                    bc = rpool.tile([128, 512], F32, tag="bc", name=f"bcs_{ic}_{p}")
                    nc.vector.tensor_copy(out=bc[:], in_=bc_ps[:])
                    at = atp.tile([128, 512], BF16, tag="at", name=f"at_{ic}_{p}")
                    ot = ots.pop((ic, p))
                    nc.vector.tensor_mul(out=at[0:64, :], in0=ot[0:64, :], in1=bc[0:64, :])
                    nc.vector.tensor_mul(out=at[64:128, :], in0=ot[64:128, :], in1=bc[64:128, :])
                    pair_ats[p] = at
                ats[ic] = pair_ats

            def emit_wo_chunk(ic, u):
                a = ats[ic]
                for n2 in range(2):
                    # alternate banks so the two matmul pairs of a chunk don't
                    # serialize behind each other's PSUM->SBUF drain
                    mk = misc_ps if n2 == 0 else L_ps
                    wo_ps = mk(f"wo_{ic}_{u}_{n2}")
                    for p in range(2):
                        nc.tensor.matmul(
                            wo_ps[:],
                            a[p][:, 128 * u : 128 * u + 128],
                            wo_sb[:, p, 512 * n2 : 512 * n2 + 512],
                            start=(p == 0),
                            stop=(p == 1),
                        )
                    if u == 3 and n2 == 1:
                        ats.pop(ic)
                    ob = obp.tile([128, 512], BF16, tag="ob", name=f"ob_{ic}_{u}_{n2}")
                    nc.vector.tensor_copy(out=ob[:], in_=wo_ps[:])
                    r0 = 512 * ic + 128 * u
                    nc.sync.dma_start(
                        out=out[r0 : r0 + 128, 512 * n2 : 512 * n2 + 512],
                        in_=ob[:],
                    )

            # ---------------- QK + ACT driver
            pts = {}

            def emit_pair(ic, p):
                qTc = qT_n[ic]
                for gi, (t0, t1) in enumerate(GROUPS):
                    pump()
                    # One score tile per key-chunk t holding BOTH heads of the
                    # pair: the two QK matmuls (row-halves) gate on the same
                    # buffer-free event, so they co-issue into the PE array.
                    tiles = []
                    for t in range(t0, t1):
                        s_t = s_ps(f"s_{ic}_{p}_{gi}_{t}")
                        kTc = kT_n[t // 4]
                        ksl = slice(128 * (t % 4), 128 * (t % 4) + 128)
                        nc.tensor.matmul(
                            s_t[:, 0, :], kTc[0:64, p, ksl], qTc[0:64, p, :],
                            start=True, stop=True,
                        )
                        nc.tensor.matmul(
                            s_t[:, 1, :], kTc[64:128, p, ksl], qTc[64:128, p, :],
                            start=True, stop=True,
                        )
                        pt_t = ptp.tile(
                            [128, 2, 512], BF16, tag="pt", name=f"pt_{ic}_{p}_{gi}_{t}"
                        )
                        nc.scalar.activation(pt_t[:], s_t[:], EXP)
                        tiles.append(pt_t)
                    pta, ptb = tiles
                    pts[(ic, p, gi)] = (pta, ptb)
                    rdy = slot[0] + 2
                    enq(rdy, lambda ic=ic, p=p, gi=gi, pta=pta, ptb=ptb:
                        emit_pv_chunk(ic, p, gi, pta, ptb))
                    if p == 1:
                        # L row-sums need both pairs' pts: foursomes per gi
                        enq(rdy, lambda ic=ic, gi=gi: emit_sums_chunk(ic, gi, pts), cost=8)
                    slot[0] += 1

            # ---------------- schedule
            # prologue: the minimum gating the first QK pair (p=0, t=0,1)
            emit_lat(0, 0)
            emit_lat(0, 1)
            emit_kt(0, 0)
            emit_qt(0, 0)

            for ic in range(4):
                base = slot[0]
                if ic == 0:
                    enq(base + 0, lambda: emit_lat(1, 0), cost=8)
                    enq(base + 0, lambda: emit_v(range(0, 2)), cost=4)
                    enq(base + 1, lambda: emit_lat(1, 1), cost=8)
                    enq(base + 1, lambda: emit_v(range(2, 4)), cost=4)
                    enq(base + 2, lambda: (emit_kt(1, 0), emit_lat(2, 0)), cost=10)
                    enq(base + 2, lambda: emit_v(range(4, 6)), cost=4)
                    enq(base + 3, lambda: (emit_lat(2, 1), emit_kt(2, 0)), cost=10)
                    enq(base + 3, lambda: emit_v(range(6, 8)), cost=4)
                    enq(base + 4, lambda: emit_lat(3, 0), cost=8)
                    enq(base + 4, lambda: emit_v(range(8, 12)), cost=8)
                    enq(base + 5, lambda: emit_lat(3, 1), cost=8)
                    enq(base + 5, lambda: emit_v(range(12, 16)), cost=8)
                    enq(base + 6, lambda: (emit_kt(3, 0), emit_qt(0, 1)), cost=10)
                    enq(base + 7, lambda: (emit_kt(0, 1), emit_kt(1, 1)), cost=4)
                    enq(base + 8, lambda: (emit_kt(2, 1), emit_kt(3, 1)), cost=4)
                    enq(base + 11, lambda: emit_qt(1, 0), cost=8)
                    enq(base + 13, lambda: emit_qt(1, 1), cost=8)
                elif ic < 3:
                    enq(base + 6, lambda n=ic + 1: emit_qt(n, 0), cost=8)
                    enq(base + 9, lambda n=ic + 1: emit_qt(n, 1), cost=8)
                emit_pair(ic, 0)
                emit_pair(ic, 1)
                enq(slot[0] + 2, lambda ic=ic: emit_norm(ic), cost=5)
                for u in range(4):
                    enq(slot[0] + 3 + 2 * u, lambda ic=ic, u=u: emit_wo_chunk(ic, u), cost=5)
            pump(drain=True)

    nc.compile()
    return nc


def _get_nc():
    if "nc" not in _STATE:
        _STATE["nc"] = _build_nc()
    return _STATE["nc"]


# ---------------------------------------------------------------- host side
def _pack_k(a, kchunks):
    """[K, N] f32/bf16 -> [128, kchunks, N] bf16 (K = 128*kchunks)."""
    K, N = a.shape
    return np.ascontiguousarray(
        np.asarray(a, np.float32).reshape(kchunks, 128, N).transpose(1, 0, 2)
    ).astype(NPBF16)


def kernel(x, Wq, bq, Wl, bl, Wk, bk, Wv, bv, Wo, bo):
    x = np.asarray(x, np.float32)
    Wq = np.asarray(Wq, np.float32)
    bq = np.asarray(bq, np.float32)
    Wl = np.asarray(Wl, np.float32)
    bl = np.asarray(bl, np.float32)
    Wk = np.asarray(Wk, np.float32)
    Wv = np.asarray(Wv, np.float32)
    bv = np.asarray(bv, np.float32)
    Wo = np.asarray(Wo, np.float32)
    bo = np.asarray(bo, np.float32)

    from concourse.bass_utils import run_bass_kernel_spmd

    trace = os.environ.get("KERNEL_TRACE", "0") == "1"
    if trace:
        _install_ntff_shim()

    wl_p = _pack_k(Wl, 8)
    bl_p = np.ascontiguousarray(bl.reshape(2, 128).T).astype(np.float32)
    in_maps = []
    for c in range(8):
        b, g = divmod(c, 4)
        sl = slice(256 * g, 256 * g + 256)
        in_maps.append(
            {
                "xT": _pack_k(x[b].T, 8),
                "wq": _pack_k(Wq[:, sl] * SCALE, 8),
                "bq": np.ascontiguousarray((bq[sl] * SCALE).reshape(2, 128).T).astype(np.float32),
                "wl": wl_p,
                "bl": bl_p,
                "wk": _pack_k(Wk[:, sl], 2),
                "wv": _pack_k(Wv[:, sl], 2),
                "wo": _pack_k(Wo[sl, :], 2),
            }
        )

    nc = _get_nc()
    res = run_bass_kernel_spmd(nc, in_maps, core_ids=list(range(8)), trace=trace)
    if trace and res.exec_time_ns is not None:
        print(f"HW exec time: {res.exec_time_ns} ns")
        _STATE["exec_time_ns"] = res.exec_time_ns

    parts = [np.asarray(res.results[c]["out"], np.float32) for c in range(8)]
    const = (bv @ Wo + bo).astype(np.float32)
    out = np.empty((2, 2048, 1024), np.float32)
    for b in range(2):
        out[b] = parts[4 * b] + parts[4 * b + 1] + parts[4 * b + 2] + parts[4 * b + 3] + const
    return out


# revision 23
# speedup vs baseline: 1.0089x; 1.0089x over previous
"""MLA attention kernel for TRN2, SPMD over 8 NeuronCores.

Sharding: core c = 4*b + g  (b = batch 0..1, g = head-group 0..3, 4 heads each).
Each core computes, for its batch b and head-group g:
    qT = (Wq_g*scale)^T x^T + bq_g*scale        [256, 2048]   (bf16)
    latT = Wl^T x^T + bl                        [256, 2048]
    kT = Wk_g^T latT                            [256, 2048]   (bk dropped: softmax shift-invariant)
    v  = latT^T Wv_g                            [2048, 256]   (bv folded into host const)
    per head h: sT = kT_h^T qT_h ; pT = exp(sT) (no max-subtraction; scores ~ N(0,1))
                l = 1^T pT ; oT = v_h^T pT ; aT = oT / l
    partial = A Wo_g                            [2048, 1024]  (f32)
Host sums the 4 partials per batch and adds (bv @ Wo + bo).

Scheduling: a slot-gated work queue keeps the scalar engine (exp ACTs, the
~154us floor) continuously fed: every QK "slot" first drains ready filler
chunks (PV, row-sum L matmuls, projections, Wo) whose inputs were emitted
>=2 slots earlier, then emits the next QK pair + exp ACTs.
PSUM: s-tiles 2bufs x 2banks, ot 2 x 1, L 1, misc 1 = 8 banks.
"""
import contextlib
import ctypes
import os
import sys
import types

if "/opt/trn_rl_repo" not in sys.path:
    sys.path.insert(0, "/opt/trn_rl_repo")

import numpy as np
import ml_dtypes

NPBF16 = ml_dtypes.bfloat16
SCALE = 64 ** -0.5
_STATE = {}


# ---------------------------------------------------------------- ntff shim
def _install_ntff_shim():
    """Provide antenv.axon_hooks so run_bass_kernel_spmd(trace=True) works."""
    if "antenv.axon_hooks" in sys.modules:
        return
    try:
        import antenv
    except ImportError:
        return

    so_path = "/opt/axon/libaxon_pjrt.so"

    def _hook_factory():
        try:
            lib = ctypes.CDLL(so_path)
        except OSError:
            return None
        if not hasattr(lib, "axon_start_nrt_profile"):
            return None
        lib.axon_start_nrt_profile.argtypes = [ctypes.POINTER(ctypes.c_int64), ctypes.c_size_t]
        lib.axon_start_nrt_profile.restype = ctypes.c_int64
        lib.axon_stop_nrt_profile.argtypes = [ctypes.c_char_p]
        lib.axon_stop_nrt_profile.restype = ctypes.c_int64

        @contextlib.contextmanager
        def _hook(output_dir, device_ids):
            import jax

            jax.devices()
            if device_ids:
                ids = (ctypes.c_int64 * len(device_ids))(*device_ids)
                rc = lib.axon_start_nrt_profile(ids, len(device_ids))
            else:
                rc = lib.axon_start_nrt_profile(None, 0)
            if rc != 0:
                raise RuntimeError(f"axon_start_nrt_profile rc={rc}")
            try:
                yield
            finally:
                n = lib.axon_stop_nrt_profile(str(output_dir).encode())
                print(f"profile: {n} file(s) written to {output_dir}", file=sys.stderr)

        return _hook

    import antenv

    mod = types.ModuleType("antenv.axon_hooks")
    _state = {"hook": _hook_factory()}
    mod.set_axon_ntff_profile_hook = lambda h: _state.__setitem__("hook", h)
    mod.get_axon_ntff_profile_hook = lambda: _state["hook"]
    sys.modules["antenv.axon_hooks"] = mod
    antenv.axon_hooks = mod


# ---------------------------------------------------------------- bass build
def _build_nc(debug_dump=False):
    import concourse.bass as bass  # noqa: F401
    import concourse.tile as tile
    from concourse import bacc, mybir

    BF16 = mybir.dt.bfloat16
    F32 = mybir.dt.float32
    EXP = mybir.ActivationFunctionType.Exp

    nc = bacc.Bacc(None, target_bir_lowering=False, debug=False)

    xT = nc.dram_tensor("xT", [128, 8, 2048], BF16, kind="ExternalInput")
    wq = nc.dram_tensor("wq", [128, 8, 256], BF16, kind="ExternalInput")
    bq = nc.dram_tensor("bq", [128, 2], F32, kind="ExternalInput")
    wl = nc.dram_tensor("wl", [128, 8, 256], BF16, kind="ExternalInput")
    bl = nc.dram_tensor("bl", [128, 2], F32, kind="ExternalInput")
    wk = nc.dram_tensor("wk", [128, 2, 256], BF16, kind="ExternalInput")
    wv = nc.dram_tensor("wv", [128, 2, 256], BF16, kind="ExternalInput")
    wo = nc.dram_tensor("wo", [128, 2, 1024], BF16, kind="ExternalInput")
    out = nc.dram_tensor("out", [2048, 1024], BF16, kind="ExternalOutput")

    GROUPS = [(2 * i, 2 * i + 2) for i in range(8)]
    GLEN = 2

    with nc.allow_low_precision("bf16 intermediates by design"), tile.TileContext(nc) as tc:
        with (
            tc.tile_pool(name="wpool", bufs=1) as wpool,
            tc.tile_pool(name="xpool", bufs=1) as xpool,
            tc.tile_pool(name="proj", bufs=1) as proj,
            tc.tile_pool(name="ptp", bufs=24) as ptp,
            tc.tile_pool(name="atp", bufs=4) as atp,
            tc.tile_pool(name="obp", bufs=4) as obp,
            tc.tile_pool(name="rpool", bufs=4) as rpool,
            tc.tile_pool(name="ps", bufs=2, space="PSUM") as ps,
        ):
            # ---------------- constants + inputs
            x_kn = [
                [xpool.tile([128, 512], BF16, name=f"x_{k}_{n}") for n in range(4)]
                for k in range(8)
            ]
            wq_sb = wpool.tile([128, 8, 256], BF16)
            wl_sb = wpool.tile([128, 8, 256], BF16)
            wk_sb = wpool.tile([128, 2, 256], BF16)
            wv_sb = wpool.tile([128, 2, 256], BF16)
            wo_sb = wpool.tile([128, 2, 1024], BF16)
            bq_sb = wpool.tile([128, 2], F32)
            bl_sb = wpool.tile([128, 2], F32)
            ones_sb = wpool.tile([128, 1], BF16)
            ones_k1 = wpool.tile([128, 64], BF16)
            nc.vector.memset(ones_sb[:], 1.0)
            nc.vector.memset(ones_k1[:], 1.0)

            # DMA order: gate the prologue (lat0/kt0/qT0) as early as possible.
            nc.sync.dma_start(out=wl_sb[:], in_=wl[:])
            nc.sync.dma_start(out=bl_sb[:], in_=bl[:])
            for k in range(8):
                nc.sync.dma_start(out=x_kn[k][0][:], in_=xT[:, k, 0:512])
            nc.sync.dma_start(out=wq_sb[:], in_=wq[:])
            nc.sync.dma_start(out=bq_sb[:], in_=bq[:])
            nc.sync.dma_start(out=wk_sb[:], in_=wk[:])
            nc.sync.dma_start(out=wv_sb[:], in_=wv[:])
            for n in range(1, 4):
                for k in range(8):
                    nc.sync.dma_start(
                        out=x_kn[k][n][:],
                        in_=xT[:, k, 512 * n : 512 * n + 512],
                    )
            nc.sync.dma_start(out=wo_sb[:], in_=wo[:])

            latT_n = [proj.tile([128, 2, 512], BF16, name=f"latT_{i}") for i in range(4)]
            qT_n = [proj.tile([128, 2, 512], BF16, name=f"qT_{i}") for i in range(4)]
            kT_n = [proj.tile([128, 2, 512], BF16, name=f"kT_{i}") for i in range(4)]
            v_sb = proj.tile([128, 16, 256], BF16)

            def s_ps(name):
                return ps.tile([128, GLEN, 512], F32, tag="s", name=name, bufs=2)

            def ot_ps(name):
                return ps.tile([128, 512], F32, tag="ot", name=name, bufs=2)

            def L_ps(name):
                return ps.tile([128, 512], F32, tag="L", name=name, bufs=1)

            def misc_ps(name):
                return ps.tile([128, 512], F32, tag="m", name=name, bufs=1)

            # HAM warm-up: small-N dummy matmuls while input DMA is in flight
            warm_sb = wpool.tile([128, 128], BF16)
            nc.vector.memset(warm_sb[:], 0.25)
            warm_ps = misc_ps("warm_ps")
            for i in range(40):
                nc.tensor.matmul(
                    warm_ps[:, 0:128], warm_sb[:], warm_sb[:],
                    start=(i == 0), stop=(i == 39),
                )

            # ---------------- slot-gated work queue
            slot = [0]
            workq = []  # (ready_slot, fn), FIFO

            def enq(ready, fn, cost=4):
                workq.append((ready, fn, cost))

            def pump(drain=False, budget=12):
                i = 0
                spent = 0
                while i < len(workq):
                    ready, fn, cost = workq[i]
                    if drain or (ready <= slot[0] and spent < budget):
                        workq.pop(i)
                        fn()
                        spent += cost
                    else:
                        i += 1

            # ---------------- projection emitters
            def emit_lat(n, m):
                acc = misc_ps(f"lat_ps_{m}_{n}")
                for k in range(8):
                    nc.tensor.matmul(
                        acc[:],
                        wl_sb[:, k, 128 * m : 128 * m + 128],
                        x_kn[k][n][:],
                        start=(k == 0),
                        stop=(k == 7),
                    )
                nc.vector.tensor_scalar_add(
                    out=latT_n[n][:, m, :], in0=acc[:], scalar1=bl_sb[:, m : m + 1]
                )

            def emit_kt(n, m):
                acc = misc_ps(f"kt_ps_{m}_{n}")
                for k in range(2):
                    nc.tensor.matmul(
                        acc[:],
                        wk_sb[:, k, 128 * m : 128 * m + 128],
                        latT_n[n][:, k, :],
                        start=(k == 0),
                        stop=(k == 1),
                    )
                nc.vector.tensor_copy(out=kT_n[n][:, m, :], in_=acc[:])

            def emit_v(ts):
                for t in ts:
                    acc = misc_ps(f"v_ps_{t}")
                    for k in range(2):
                        nc.tensor.matmul(
                            acc[:, 0:256],
                            latT_n[t // 4][:, k, 128 * (t % 4) : 128 * (t % 4) + 128],
                            wv_sb[:, k, :],
                            start=(k == 0),
                            stop=(k == 1),
                        )
                    nc.vector.tensor_copy(out=v_sb[:, t, :], in_=acc[:, 0:256])

            def emit_qt(n, m):
                acc = misc_ps(f"q_ps_{m}_{n}")
                for k in range(8):
                    nc.tensor.matmul(
                        acc[:],
                        wq_sb[:, k, 128 * m : 128 * m + 128],
                        x_kn[k][n][:],
                        start=(k == 0),
                        stop=(k == 7),
                    )
                nc.vector.tensor_scalar_add(
                    out=qT_n[n][:, m, :], in0=acc[:], scalar1=bq_sb[:, m : m + 1]
                )

            # ---------------- attention pieces
            ots = {}   # (ic, p) -> ot psum tile
            Ls = {}    # ic -> L psum tile
            ats = {}   # ic -> {p: at tile}

            def emit_pv_chunk(ic, p, gi, pta, ptb):
                """pta/ptb = pt tiles for key-chunks t0/t1, each [128, 2heads, 512]."""
                h0, h1 = 2 * p, 2 * p + 1
                if gi == 0:
                    ots[(ic, p)] = ot_ps(f"ot_{ic}_{p}")
                ot0 = ots[(ic, p)]
                t0, t1 = GROUPS[gi]
                for t in range(t0, t1):
                    pt = pta if t == t0 else ptb
                    nc.tensor.matmul(
                        ot0[0:64, :], v_sb[:, t, 64 * h0 : 64 * h0 + 64], pt[:, 0, :],
                        start=(t == 0), stop=(t == 15), skip_group_check=True,
                    )
                    nc.tensor.matmul(
                        ot0[64:128, :], v_sb[:, t, 64 * h1 : 64 * h1 + 64], pt[:, 1, :],
                        start=(t == 0), stop=(t == 15), skip_group_check=True,
                    )

            def emit_sums_chunk(ic, gi, pts):
                """L row-sums for all 4 heads (both pairs) of group gi —
                foursomes of col tiles issue concurrently."""
                if gi == 0:
                    L = L_ps(f"L_{ic}")
                    nc.vector.memset(L[:], 1.0)
                    Ls[ic] = L
                L = Ls[ic]
                t0, t1 = GROUPS[gi]
                for t in range(t0, t1):
                    tt = t - t0
                    for p in range(2):
                        pta, ptb = pts[(ic, p, gi)]
                        pt = pta if t == t0 else ptb
                        for j, hh in enumerate((2 * p, 2 * p + 1)):
                            nc.tensor.matmul(
                                L[32 * hh : 32 * hh + 1, :],
                                ones_sb[:],
                                pt[:, j, :],
                                start=(t == 0),
                                stop=(t == 15),
                                tile_position=(0, 32 * hh),
                                skip_group_check=True,
                            )
                if gi == 7:
                    for p in range(2):
                        for g2 in range(8):
                            pts.pop((ic, p, g2))

            def emit_norm(ic):
                L = Ls.pop(ic)
                recip = rpool.tile([128, 512], BF16, tag="recip", name=f"recip_{ic}")
                nc.vector.reciprocal(out=recip[:], in_=L[:])
                pair_ats = {}
                for p in range(2):
                    bc_ps = misc_ps(f"bcp_{ic}_{p}")
                    for j, hh in enumerate((2 * p, 2 * p + 1)):
                        rb = 32 * hh
                        nc.tensor.matmul(
                            bc_ps[64 * j : 64 * j + 64, :],
                            ones_k1[rb : rb + 1, 0:64],
                            recip[rb : rb + 1, :],
                            start=True,
                            stop=True,
                            tile_position=(rb, 64 * j),
                            skip_group_check=True,
                        )
                    bc = rpool.tile([128, 512], F32, tag="bc", name=f"bcs_{ic}_{p}")
                    nc.vector.tensor_copy(out=bc[:], in_=bc_ps[:])
                    at = atp.tile([128, 512], BF16, tag="at", name=f"at_{ic}_{p}")
                    ot = ots.pop((ic, p))
                    nc.vector.tensor_mul(out=at[0:64, :], in0=ot[0:64, :], in1=bc[0:64, :])
                    nc.vector.tensor_mul(out=at[64:128, :], in0=ot[64:128, :], in1=bc[64:128, :])
                    pair_ats[p] = at
                ats[ic] = pair_ats

            def emit_wo_chunk(ic, u):
                a = ats[ic]
                for n2 in range(2):
                    # alternate banks so the two matmul pairs of a chunk don't
                    # serialize behind each other's PSUM->SBUF drain
                    mk = misc_ps if n2 == 0 else L_ps
                    wo_ps = mk(f"wo_{ic}_{u}_{n2}")
                    for p in range(2):
                        nc.tensor.matmul(
                            wo_ps[:],
                            a[p][:, 128 * u : 128 * u + 128],
                            wo_sb[:, p, 512 * n2 : 512 * n2 + 512],
                            start=(p == 0),
                            stop=(p == 1),
                        )
                    if u == 3 and n2 == 1:
                        ats.pop(ic)
                    ob = obp.tile([128, 512], BF16, tag="ob", name=f"ob_{ic}_{u}_{n2}")
                    nc.vector.tensor_copy(out=ob[:], in_=wo_ps[:])
                    r0 = 512 * ic + 128 * u
                    nc.sync.dma_start(
                        out=out[r0 : r0 + 128, 512 * n2 : 512 * n2 + 512],
                        in_=ob[:],
                    )

            # ---------------- QK + ACT driver
            pts = {}

            def emit_pair(ic, p):
                qTc = qT_n[ic]
                for gi, (t0, t1) in enumerate(GROUPS):
                    pump()
                    # One score tile per key-chunk t holding BOTH heads of the
                    # pair: the two QK matmuls (row-halves) gate on the same
                    # buffer-free event, so they co-issue into the PE array.
                    tiles = []
                    for t in range(t0, t1):
                        s_t = s_ps(f"s_{ic}_{p}_{gi}_{t}")
                        kTc = kT_n[t // 4]
                        ksl = slice(128 * (t % 4), 128 * (t % 4) + 128)
                        nc.tensor.matmul(
                            s_t[:, 0, :], kTc[0:64, p, ksl], qTc[0:64, p, :],
                            start=True, stop=True,
                        )
                        nc.tensor.matmul(
                            s_t[:, 1, :], kTc[64:128, p, ksl], qTc[64:128, p, :],
                            start=True, stop=True,
                        )
                        pt_t = ptp.tile(
                            [128, 2, 512], BF16, tag="pt", name=f"pt_{ic}_{p}_{gi}_{t}"
                        )
                        nc.scalar.activation(pt_t[:], s_t[:], EXP)
                        tiles.append(pt_t)
                    pta, ptb = tiles
                    pts[(ic, p, gi)] = (pta, ptb)
                    rdy = slot[0] + 2
                    enq(rdy, lambda ic=ic, p=p, gi=gi, pta=pta, ptb=ptb:
                        emit_pv_chunk(ic, p, gi, pta, ptb))
                    if p == 1:
                        # L row-sums need both pairs' pts: foursomes per gi
                        enq(rdy, lambda ic=ic, gi=gi: emit_sums_chunk(ic, gi, pts), cost=8)
                    slot[0] += 1

            # ---------------- schedule
            # prologue: the minimum gating the first QK pair (p=0, t=0,1)
            emit_lat(0, 0)
            emit_lat(0, 1)
            emit_kt(0, 0)
            emit_qt(0, 0)

            for ic in range(4):
                base = slot[0]
                if ic == 0:
                    enq(base + 0, lambda: (emit_lat(1, 0), emit_lat(1, 1), emit_kt(1, 0)), cost=18)
                    enq(base + 1, lambda: (emit_lat(2, 0), emit_lat(2, 1), emit_kt(2, 0), emit_v(range(0, 4))), cost=26)
                    enq(base + 2, lambda: (emit_lat(3, 0), emit_lat(3, 1), emit_kt(3, 0), emit_v(range(4, 8))), cost=26)
                    enq(base + 3, lambda: (emit_kt(0, 1), emit_kt(1, 1), emit_qt(0, 1)), cost=12)
                    enq(base + 4, lambda: (emit_kt(2, 1), emit_kt(3, 1), emit_v(range(8, 12))), cost=12)
                    enq(base + 5, lambda: emit_v(range(12, 16)), cost=8)
                if ic < 3:
                    enq(base + 6, lambda n=ic + 1: emit_qt(n, 0), cost=8)
                    enq(base + 9, lambda n=ic + 1: emit_qt(n, 1), cost=8)
                emit_pair(ic, 0)
                emit_pair(ic, 1)
                enq(slot[0] + 2, lambda ic=ic: emit_norm(ic), cost=5)
                for u in range(4):
                    enq(slot[0] + 3 + 2 * u, lambda ic=ic, u=u: emit_wo_chunk(ic, u), cost=5)
            pump(drain=True)

    nc.compile()
    return nc


def _get_nc():
    if "nc" not in _STATE:
        _STATE["nc"] = _build_nc()
    return _STATE["nc"]


# ---------------------------------------------------------------- host side
def _pack_k(a, kchunks):
    """[K, N] f32/bf16 -> [128, kchunks, N] bf16 (K = 128*kchunks)."""
    K, N = a.shape
    return np.ascontiguousarray(
        np.asarray(a, np.float32).reshape(kchunks, 128, N).transpose(1, 0, 2)
    ).astype(NPBF16)


def kernel(x, Wq, bq, Wl, bl, Wk, bk, Wv, bv, Wo, bo):
    x = np.asarray(x, np.float32)
    Wq = np.asarray(Wq, np.float32)
    bq = np.asarray(bq, np.float32)
    Wl = np.asarray(Wl, np.float32)
    bl = np.asarray(bl, np.float32)
    Wk = np.asarray(Wk, np.float32)
    Wv = np.asarray(Wv, np.float32)
    bv = np.asarray(bv, np.float32)
    Wo = np.asarray(Wo, np.float32)
    bo = np.asarray(bo, np.float32)

    from concourse.bass_utils import run_bass_kernel_spmd

    trace = os.environ.get("KERNEL_TRACE", "0") == "1"
    if trace:
        _install_ntff_shim()

    wl_p = _pack_k(Wl, 8)
    bl_p = np.ascontiguousarray(bl.reshape(2, 128).T).astype(np.float32)
    in_maps = []
    for c in range(8):
        b, g = divmod(c, 4)
        sl = slice(256 * g, 256 * g + 256)
        in_maps.append(
            {
                "xT": _pack_k(x[b].T, 8),
                "wq": _pack_k(Wq[:, sl] * SCALE, 8),
                "bq": np.ascontiguousarray((bq[sl] * SCALE).reshape(2, 128).T).astype(np.float32),
                "wl": wl_p,
                "bl": bl_p,
                "wk": _pack_k(Wk[:, sl], 2),
                "wv": _pack_k(Wv[:, sl], 2),
                "wo": _pack_k(Wo[sl, :], 2),
            }
        )

    nc = _get_nc()
    res = run_bass_kernel_spmd(nc, in_maps, core_ids=list(range(8)), trace=trace)
    if trace and res.exec_time_ns is not None:
        print(f"HW exec time: {res.exec_time_ns} ns")
        _STATE["exec_time_ns"] = res.exec_time_ns

    parts = [np.asarray(res.results[c]["out"], np.float32) for c in range(8)]
    const = (bv @ Wo + bo).astype(np.float32)
    out = np.empty((2, 2048, 1024), np.float32)
    for b in range(2):
        out[b] = parts[4 * b] + parts[4 * b + 1] + parts[4 * b + 2] + parts[4 * b + 3] + const
    return out
